# revision 38
# baseline (speedup 1.0000x reference)
"""AttentionBlock (GroupNorm + single-head 4096x4096 attention + proj + residual)
on 8 Trainium2 NeuronCores.

Sharding: core c = 2*b + h handles image b (of 4), query-half h (of 2).
Each core:
  - receives x pre-transposed to channel-major bf16 (host does the transpose),
  - computes GroupNorm statistics via bn_stats while x streams in,
  - computes kT [512,4096] and v [4096,512] for the full image (k/v duplicated
    across the half-pair, ~10% extra FLOPs, no collectives),
  - computes qT for its 2048 query rows,
  - attention over its 2048 queries, projection + bias + residual for its rows.

Precision: fp8e4m3 (TRN 240-max) with MatmulPerfMode.DoubleRow for ALL GEMMs
(hn/q/k/v/scores/PV/rowsum/proj), fp32 PSUM accumulation, fp32 GroupNorm
statistics, fp32 softmax row-sums / normalization, fp32 residual.

fp8 scale bookkeeping:
  - wq/wk/wv/wp host-scaled x16 (keeps N(0, 1/sqrt(C)) weights out of the fp8
    subnormal range); biases bq/bk/bv host-scaled x16 to match.
  - qT/kT hold 16q/16k; scores PSUM = 256*q.k; exp scale folds the 1/256.
  - exp has bias -2.0 (max score 6.81 -> et max e^4.81=123 < 240 fp8 max);
    the e^-2 factor cancels between numerator and row-sum.
  - vS holds 16v; po PSUM = 16*sum(et*v); ot eviction scales by 1/128.
  - proj PSUM py = (po/128) @ (16 wp) = 2*sum(et*v)@wp; rowsum matmul uses a
    2.0-valued ones vector so rt = 1/pr = 1/(2*sum(et)) normalizes exactly.
"""

import sys

sys.path.insert(0, "/opt/trn_rl_repo")

import numpy as np  # noqa: E402

import bass_rust  # noqa: E402
import concourse.bass as bass  # noqa: E402
import concourse.mybir as mybir  # noqa: E402
import concourse.tile as tile  # noqa: E402
from concourse.vector_clock import ScopedClock  # noqa: E402
from concourse.bass_utils import run_bass_kernel_spmd  # noqa: E402

F32 = mybir.dt.float32
BF16 = mybir.dt.bfloat16
F8 = mybir.dt.float8e4
AF = mybir.ActivationFunctionType
OP = mybir.AluOpType
DR = mybir.MatmulPerfMode.DoubleRow

B, H, W, C = 4, 64, 64, 512
HW = H * W            # 4096 positions per image
HALF = HW // 2        # 2048 query rows per core
GROUPS = 32
GSIZE = C // GROUPS   # 16 channels per group
EPS = 1e-5
N_CORES = 8
CT = C // 128         # 4 channel partition-tiles
JT = HW // 128        # 32 position partition-tiles
JC = HW // 512        # 8 position chunks (kT/v build)
QC = HALF // 512      # 4 query chunks (qT build)
IB = HALF // 512      # 4 query i-blocks (attention)
WSC = 16.0            # host-side weight/bias scale (fp8 subnormal avoidance)
SM8 = 1.0 / (WSC * WSC * float(np.sqrt(C)))   # exp scale on (16q).(16k) psum
EXPB = -2.0           # exp bias: keeps et = e^(s-2) <= e^4.9 < 240 (fp8 max)
OTS = 1.0 / 128.0     # po -> ot eviction scale (fp8 range)
ONESV = 2.0           # rowsum weights: pr = 2*sum(et) so rt=1/pr normalizes
                      # py = (po/128)@(16wp) = 2*sum(et*v)@wp exactly


# --- workaround: walrus in this container rejects instructions carrying more
# than one sync-wait command.  Move extra waits onto same-engine NOPs placed
# immediately before the instruction (engine program order makes this exact).
def _split_multi_waits(nc, max_waits=1):
    n = 0
    for f in nc.m.functions:
        for bb in f.blocks:
            newlist = []
            for inst in bb.instructions:
                si = inst.sync_info
                waits = list(si.on_wait) if si is not None else []
                if len(waits) > max_waits:
                    n += 1
                    for k, wt in enumerate(waits[:-max_waits]):
                        nop = bass_rust.InstNoOp(
                            name=f"{inst.name}-sw{k}", engine=inst.engine)
                        nop.sync_info = mybir.SyncInfo(on_wait=[wt], on_update=[])
                        newlist.append(nop)
                    inst.sync_info = mybir.SyncInfo(
                        on_wait=waits[-max_waits:], on_update=list(si.on_update))
                newlist.append(inst)
            bb.instructions[:] = newlist
    return n


def _split_drain_and_barrier(self, tick_clock, wait_clock):
    # same as TileContext._drain_and_barrier but with the tail drain's waits
    # split onto single-wait NOPs (same walrus limitation as above).
    drain_inst = self.nc.sync.drain()
    wait_clock.add_sem_waits(
        drain_inst.ins, ScopedClock({None: tick_clock.global_clock}))
    mi = drain_inst.ins
    waits = list(mi.sync_info.on_wait) if mi.sync_info is not None else []
    if len(waits) > 1:
        mi.sync_info.on_wait = []
        for wt in waits:
            wi = self.nc.sync.nop(nofuse=True, hint="tail_drain_wait")
            wi.ins.sync_info = mybir.SyncInfo(on_wait=[wt], on_update=[])
    self.nc.all_engine_barrier()
    assert self.sems is not None
    popped = self.nc._tile_sem_poison_stack.pop()
    assert popped is self._sem_poison
    self.nc.clear_and_free_semaphores(list(self.sems.allocated().values()))
    self.nc.all_engine_barrier()


tile.TileContext._drain_and_barrier = _split_drain_and_barrier


def build_program(split_waits=True):
    nc = bass.Bass()

    # xT rows are permuted per-core so the query half is always positions
    # [0, HALF), host-transposed to channel-major and slab-tiled
    # [2, CT, 128, 2048] fp8 so each (half, ct) slab DMA is one contiguous
    # 256KB read -- the x stream gates the GroupNorm stats chain, so halving
    # its bytes (vs bf16) shortens the serial startup.  fp8 x only perturbs
    # hn by ~3% (already the hnT quantization level) and GN variance by 0.1%.
    xTd = nc.dram_tensor("xT", [2, CT, 128, 2048], F8, kind="ExternalInput")
    xq = nc.dram_tensor("xq", [HALF, C], F32, kind="ExternalInput")
    # weights pre-cast to fp8 (x16) and pre-tiled [128, CT, C] on the host --
    # one fewer rounding than the old bf16->fp8 device cast, half the DMA
    # bytes, and no scalar-engine casts clogging the queue ahead of the GN
    # sqrt (which stalled phase B by ~7us).
    wq = nc.dram_tensor("wq", [128, CT, C], F8, kind="ExternalInput")
    wk = nc.dram_tensor("wk", [128, CT, C], F8, kind="ExternalInput")
    wv = nc.dram_tensor("wv", [128, CT, C], F8, kind="ExternalInput")
    wp = nc.dram_tensor("wp", [128, CT, C], F8, kind="ExternalInput")
    # packed per-channel constants [128, CT, 4] = (16*bq, 16*bk, gamma, beta)
    cvecd = nc.dram_tensor("cvec", [128, CT, 4], F32, kind="ExternalInput")
    # bp here is host-computed bp + bv @ wp (bv folded through the attention)
    bpd = nc.dram_tensor("bp", [C], F32, kind="ExternalInput")
    gseld = nc.dram_tensor("gsel", [GROUPS, C], F32, kind="ExternalInput")
    # gsel2[p, ct, g] = 1/GSIZE where channel ct*128+p belongs to group g
    gsel2d = nc.dram_tensor("gsel2", [128, CT, GROUPS], F32, kind="ExternalInput")
    # y in bf16: halves the output DMA on the tail critical path; the bf16
    # rounding of x + proj adds <= 2^-9 relative error (~2e-3, budget 2e-2)
    yd = nc.dram_tensor("y", [HALF, C], BF16, kind="ExternalOutput")

    xqt = xq[:, :].rearrange("(t p) c -> t p c", p=128)   # [16,128,512]
    yt = yd[:, :].rearrange("(t p) c -> t p c", p=128)    # [16,128,512]

    with tile.TileContext(nc) as tc:
        # ---------------- persistent storage + constants ----------------
        store = tc.alloc_tile_pool(name="store", bufs=1)
        kT = store.tile([128, CT, HW], F8)       # kT[c%128, c//128, j] = 16k
        vS = store.tile([128, JT, C], F8)        # v[j%128, j//128, c] = 16v
        qT = store.tile([128, CT, HALF], F8)     # qT[c%128, c//128, i] = 16q
        # x^T in fp8: one tile [cin%128, cin//128, half, j-in-half] so QKV
        # matmul moving APs can span ct-pairs (DoubleRow contraction)
        xTa = store.tile([128, CT, 2, 2048], F8)
        wqr = store.tile([128, CT, C], F8)       # 16*wq, [cin%128, cin//128, cout]
        wkr = store.tile([128, CT, C], F8)
        wvr = store.tile([128, CT, C], F8)
        wpr = store.tile([128, CT, C], F8)
        # GN-scale-folded weights: wS = diag(s) @ (16*w), built in phase A
        wqS = store.tile([128, CT, C], F8)
        wkS = store.tile([128, CT, C], F8)
        wvS = store.tile([128, CT, C], F8)
        # 256*t and 256*(t^T wv) as DR weights (col 0; pair-dim step 16B)
        t8f = store.tile([128, CT, 16], F8)
        twv8f = store.tile([128, CT, 16], F8)
        cst = tc.alloc_tile_pool(name="cst", bufs=1)
        gsel = cst.tile([GROUPS, C], F32)
        gsel2 = cst.tile([128, CT, GROUPS], F32)
        # [128, 2, 16] so the DoubleRow weight AP's pair-dim step is 16 bytes
        # (walrus s3_lw_dual_fp8_restrictions requires step % 16 == 0)
        ones2 = cst.tile([128, 2, 16], F8)
        nc.vector.memset(ones2, ONESV)
        expb = cst.tile([128, 1], F32)
        nc.vector.memset(expb, EXPB)
        # DRAM scratch to re-layout softmax row-sums [1,512] -> [128,4]
        # (two halves per i-block: the n<8 half bounces mid-block so only
        # the second half's round-trip sits on the tail critical path)
        sumscr = nc.dram_tensor("sumscr", [IB, 2, 512], F32)
        # DRAM scratch to re-layout t^T w rank-1 results [1,512] -> [128,4]
        twscr = nc.dram_tensor("twscr", [4, 512], F32)
        cv = cst.tile([128, CT, 4], F32)   # (16bq, 16bk, gamma, beta)
        bp_bc = cst.tile([128, C], F32)
        s_sb = cst.tile([128, CT], F32)   # GN scale per channel
        t_sb = cst.tile([128, CT], F32)   # GN shift per channel
        # eviction biases with the GN t-term folded in: 16(bq + t^T wq) etc.
        cvq2 = cst.tile([128, CT], F32)
        cvk2 = cst.tile([128, CT], F32)
        bp_bc2 = cst.tile([128, C], F32)   # bp + bv@wp + (t^T wv)@wp

        # 8 x-slab DMAs (512KB contiguous each) FIRST, spread over the three
        # DMA-capable queues (two queues topped out at ~200GB/s; 4MB of x
        # gates the GroupNorm stats chain), ct-major so the per-ct stats
        # aggregation pipelines behind the DMAs.  Weights + consts after.
        xengs = [nc.sync, nc.gpsimd, nc.scalar]
        for ct in range(CT):
            for hf in range(2):
                xengs[(2 * ct + hf) % 3].dma_start(
                    out=xTa[:, ct, hf, :], in_=xTd[hf, ct, :, :])
        nc.sync.dma_start(out=gsel2, in_=gsel2d[:, :, :])
        nc.sync.dma_start(out=cv, in_=cvecd[:, :, :])
        nc.sync.dma_start(out=wkr, in_=wk[:, :, :])
        nc.sync.dma_start(out=wpr, in_=wp[:, :, :])
        nc.scalar.dma_start(out=gsel, in_=gseld[:, :])
        nc.scalar.dma_start(out=bp_bc, in_=bpd[:].partition_broadcast(128))
        nc.scalar.dma_start(out=wvr, in_=wv[:, :, :])
        nc.gpsimd.dma_start(out=wqr, in_=wq[:, :, :])

        # ------- phase A: GroupNorm stats as the slabs land (no PE work) ----
        # ct-major: each ct's stats aggregate + feed the group matmul while
        # the next ct's slabs are still streaming in, shortening the serial
        # chain after the last bn_stats.
        with tc.tile_pool(name="pa_small", bufs=1) as pas, \
             tc.tile_pool(name="pa_ps", bufs=2, space="PSUM") as pa_ps:
            stats_sb = pas.tile([128, CT, JC, 6], F32)
            # full-array DR warm weights, written AT the stats-chain marker
            # points (data-dependence gates each warm block's start)
            wgates = [pas.tile([128, 2, 128], F8, tag=f"wg{i}", name=f"wg{i}")
                      for i in range(3)]
            epst = pas.tile([GROUPS, 1], F32)
            nc.vector.memset(epst, EPS)
            g2 = pa_ps.tile([GROUPS, 2], F32, tag="gagg")
            mv_all = pas.tile([128, CT, 2], F32)
            sp_all = pas.tile([128, CT, 2], F32)
            for ct in range(CT):
                for jc in range(JC):
                    hf, sc = jc // 4, (jc % 4) * 512
                    nc.vector.bn_stats(
                        out=stats_sb[:, ct, jc, :],
                        in_=xTa[:, ct, hf, sc:sc + 512])
                    if ct == 0 and jc == 0:
                        # gate warm block 0 on the very first stats tile
                        nc.vector.tensor_scalar(
                            wgates[0][:, :, :], xTa[:, 0:2, 0, 0:128],
                            stats_sb[:, 0, 0, 1:2], None, OP.mult)
                nc.vector.bn_aggr(out=mv_all[:, ct, :], in_=stats_sb[:, ct, :, :])
                nc.vector.tensor_mul(sp_all[:, ct, 0:1], mv_all[:, ct, 0:1],
                                     mv_all[:, ct, 0:1])
                nc.vector.tensor_add(sp_all[:, ct, 1:2], sp_all[:, ct, 0:1],
                                     mv_all[:, ct, 1:2])
                nc.vector.tensor_copy(sp_all[:, ct, 0:1], mv_all[:, ct, 0:1])
                nc.tensor.matmul(g2[:, :], gsel2[:, ct, :], sp_all[:, ct, :],
                                 start=(ct == 0), stop=(ct == CT - 1))
                if ct == 2:
                    # mid-chain marker; warm matmuls MUST be full-array: HAM
                    # gates the clock on PE array power, and a 1-column warm
                    # matmul reads as idle, re-throttling to 1.2GHz right
                    # before phase B.
                    nc.vector.tensor_scalar(
                        wgates[1][:, :, :], xTa[:, 0:2, 0, 0:128],
                        stats_sb[:, ct, 0, 1:2], None, OP.mult)
            nc.vector.tensor_scalar(
                wgates[2][:, :, :], xTa[:, 0:2, 0, 0:128],
                stats_sb[:, 3, 0, 1:2], None, OP.mult)
            with tc.tile_pool(name="pa_warm", bufs=1, space="PSUM") as pwm:
                pw = pwm.tile([128, 512], F32)
                for wi, reps in ((0, 48), (1, 18), (2, 21)):
                    for r in range(reps):
                        nc.tensor.matmul(
                            pw[:, :], wgates[wi][:, :, :],
                            xTa[:, 0:2, 0, 0:512],
                            start=True, stop=True, perf_mode=DR)

            if True:
                # group mean/var -> (mean, rstd); keep the serial chain on the
                # vector engine (one scalar hop for sqrt) -- cross-engine hops
                # cost ~0.5-1us each in queue + semaphore latency
                mv2 = pas.tile([GROUPS, 2], F32)
                nc.vector.tensor_copy(mv2[:, :], g2[:, :])   # (mean, E[x^2])
                var = pas.tile([GROUPS, 1], F32)
                nc.vector.tensor_mul(var[:, :], mv2[:, 0:1], mv2[:, 0:1])
                nc.vector.tensor_sub(var[:, :], mv2[:, 1:2], var[:, :])
                sd = pas.tile([GROUPS, 1], F32)
                nc.scalar.activation(sd[:, :], var[:, :], AF.Sqrt, bias=epst[:, :])
                nc.vector.reciprocal(mv2[:, 1:2], sd[:, :])
                # broadcast group (mean, rstd) to channels into ONE psum tile,
                # one eviction, then s/t (batched on vector)
                bc_all = pas.tile([128, CT, 2], F32)
                pbc = pa_ps.tile([128, CT, 2], F32, tag="bcast")
                for ct in range(CT):
                    nc.tensor.matmul(pbc[:, ct, :], gsel[:, ct * 128:(ct + 1) * 128],
                                     mv2[:, :], start=True, stop=True)
                nc.vector.tensor_copy(bc_all[:, :, :], pbc[:, :, :])
                nc.vector.tensor_mul(s_sb[:, :], cv[:, :, 2], bc_all[:, :, 1])
                tmp = pas.tile([128, CT], F32)
                nc.vector.tensor_mul(tmp[:, :], bc_all[:, :, 0], s_sb[:, :])
                nc.vector.tensor_sub(t_sb[:, :], cv[:, :, 3], tmp[:, :])
                # fold the GN scale into the QKV weights (wS = diag(s) @ 16w):
                # k = (diag(s) 16wk)^T x + 16(t^T wk + bk).  This removes the
                # per-position hn normalization pass entirely -- phase B's
                # GEMMs consume the raw fp8 x slabs.  Split across the three
                # elementwise engines; wkS first (kT GEMMs consume it first).
                # 256*t as a DR weight column (rank-1 t^T w matmuls below;
                # first on vector -- the t^T w matmuls are the PE's first
                # real work and gate the eviction-bias chain)
                nc.vector.tensor_scalar(t8f[:, :, 0], t_sb[:, :],
                                        256.0, None, OP.mult)
                # gpsimd lacks TensorScalarPtr, so the 12 scale ops split
                # vector/scalar in consumption order (K, Q, then V); keeping
                # vector's tail short matters -- phase B's PSUM evictions
                # queue behind it, and a clogged vector queue starves the PE
                # of free PSUM banks.
                for ct in range(CT):
                    nc.vector.tensor_scalar(wkS[:, ct, :], wkr[:, ct, :],
                                            s_sb[:, ct:ct + 1], None, OP.mult)
                    nc.scalar.activation(wqS[:, ct, :], wqr[:, ct, :],
                                         AF.Copy, scale=s_sb[:, ct:ct + 1])
                for ct in range(CT):
                    if ct < 2:
                        nc.vector.tensor_scalar(wvS[:, ct, :], wvr[:, ct, :],
                                                s_sb[:, ct:ct + 1], None,
                                                OP.mult)
                    else:
                        nc.scalar.activation(wvS[:, ct, :], wvr[:, ct, :],
                                             AF.Copy,
                                             scale=s_sb[:, ct:ct + 1])

        # ---------------- phase B: K,V (and Q) GEMMs on raw x ---------------
        # eviction engines alternate scalar/vector (gpsimd cannot read PSUM)
        _ev = [0]

        def ev_out(out, pin, bias):
            e = _ev[0] % 2
            _ev[0] += 1
            if bias is None:
                if e == 0:
                    nc.scalar.activation(out, pin, AF.Copy)
                else:
                    nc.vector.tensor_copy(out, pin)
            else:
                if e == 0:
                    nc.scalar.activation(out, pin, AF.Identity, bias=bias)
                else:
                    nc.vector.tensor_scalar(out, pin, bias, None, OP.add)

        def qkv_chunk(pb_ps, jc):
            hf, js = jc // 4, (jc % 4) * 512
            for ct in range(CT):
                pk = pb_ps.tile([128, 512], F32, tag="qkv")
                for k2 in range(2):
                    nc.tensor.matmul(
                        pk[:, :], wkS[:, 2 * k2:2 * k2 + 2, ct * 128:(ct + 1) * 128],
                        xTa[:, 2 * k2:2 * k2 + 2, hf, js:js + 512],
                        start=(k2 == 0), stop=(k2 == 1), perf_mode=DR)
                ev_out(kT[:, ct, jc * 512:(jc + 1) * 512], pk[:, :],
                       cvk2[:, ct:ct + 1])
            if jc < QC:   # rows [0, HALF) are the query rows
                for ct in range(CT):
                    pq = pb_ps.tile([128, 512], F32, tag="qkv")
                    for k2 in range(2):
                        nc.tensor.matmul(
                            pq[:, :], wqS[:, 2 * k2:2 * k2 + 2, ct * 128:(ct + 1) * 128],
                            xTa[:, 2 * k2:2 * k2 + 2, hf, js:js + 512],
                            start=(k2 == 0), stop=(k2 == 1), perf_mode=DR)
                    ev_out(qT[:, ct, jc * 512:(jc + 1) * 512], pq[:, :],
                           cvq2[:, ct:ct + 1])
            for jp in range(4):
                pv = pb_ps.tile([128, 512], F32, tag="qkv")
                for k2 in range(2):
                    nc.tensor.matmul(
                        pv[:, :],
                        xTa[:, 2 * k2:2 * k2 + 2, hf, js + jp * 128:js + (jp + 1) * 128],
                        wvS[:, 2 * k2:2 * k2 + 2, :],
                        start=(k2 == 0), stop=(k2 == 1), perf_mode=DR)
                # bv is folded into bp on the host (softmax rows sum to 1 so
                # attn(v + bv) = attn(v) + bv exactly); the GN t-term rides
                # the same way via bp_bc2.  Eviction is a pure cast.
                ev_out(vS[:, jc * 4 + jp, :], pv[:, :], None)

        with tc.tile_pool(name="pb_ps", bufs=7, space="PSUM") as pb_ps, \
             tc.tile_pool(name="tw_ps", bufs=1, space="PSUM") as tw_ps, \
             tc.tile_pool(name="tw_sb", bufs=4) as tw_sb:
            # rank-1 t^T w matmuls (one 216ns matmul each; evict via scalar
            # copy -> DRAM bounce -> [128, CT] relayout on the gpsimd queue),
            # interleaved with the first QKV chunks so the PE never idles.
            def twmm(wr, row):
                ptw = tw_ps.tile([1, 512], F32, tag="tw")
                for k2 in range(2):
                    nc.tensor.matmul(
                        ptw[:, :], t8f[:, 2 * k2:2 * k2 + 2, 0:1],
                        wr[:, 2 * k2:2 * k2 + 2, :],
                        start=(k2 == 0), stop=(k2 == 1), perf_mode=DR)
                stg = tw_sb.tile([1, 512], F32, tag=f"twe{row}")
                nc.scalar.activation(stg[:, :], ptw[:, :], AF.Copy)
                nc.gpsimd.dma_start(out=twscr[row:row + 1, :], in_=stg[:, :])

            def twback(row, out4):
                nc.gpsimd.dma_start(
                    out=out4,
                    in_=twscr[row, :].rearrange("(b p) -> p b", p=128))

            twmm(wkr, 0)
            twmm(wqr, 1)
            twmm(wvr, 2)
            twk4 = cst.tile([128, CT], F32)
            twq4 = cst.tile([128, CT], F32)
            twv4 = cst.tile([128, CT], F32)
            twback(0, twk4[:, :])
            twback(1, twq4[:, :])
            # 16(bk + t^T wk) = 16bk + psum/256   (psum = 256t . 16wk)
            nc.vector.scalar_tensor_tensor(
                cvk2[:, :], twk4[:, :], 1.0 / 256.0, cv[:, :, 1],
                OP.mult, OP.add)
            nc.vector.scalar_tensor_tensor(
                cvq2[:, :], twq4[:, :], 1.0 / 256.0, cv[:, :, 0],
                OP.mult, OP.add)
            qkv_chunk(pb_ps, 0)
            twback(2, twv4[:, :])
            # 256*(t^T wv) in fp8 for the second rank-1 hop through wp
            nc.vector.tensor_scalar(twv8f[:, :, 0], twv4[:, :],
                                    1.0 / 16.0, None, OP.mult)
            qkv_chunk(pb_ps, 1)
            qkv_chunk(pb_ps, 2)
            # (t^T wv) @ wp -> broadcast into bp_bc2 (rides the softmax like
            # bv: attn rows sum to 1, so it lands as a per-cout constant)
            ptv = tw_ps.tile([1, 512], F32, tag="tw")
            for k2 in range(2):
                nc.tensor.matmul(
                    ptv[:, :], twv8f[:, 2 * k2:2 * k2 + 2, 0:1],
                    wpr[:, 2 * k2:2 * k2 + 2, :],
                    start=(k2 == 0), stop=(k2 == 1), perf_mode=DR)
            stv = tw_sb.tile([1, 512], F32, tag="twe3")
            nc.scalar.activation(stv[:, :], ptv[:, :], AF.Copy)
            nc.gpsimd.dma_start(out=twscr[3:4, :], in_=stv[:, :])
            tvpb = tw_sb.tile([128, C], F32, tag="tvpb")
            nc.gpsimd.dma_start(
                out=tvpb[:, :], in_=twscr[3, :].partition_broadcast(128))
            nc.vector.scalar_tensor_tensor(
                bp_bc2[:, :], tvpb[:, :], 1.0 / 4096.0, bp_bc[:, :],
                OP.mult, OP.add)
            for jc in range(3, JC):
                qkv_chunk(pb_ps, jc)

        # ---------------- phase C: attention + projection + residual --------
        with tc.tile_pool(name="pc_sb", bufs=4) as pcs, \
             tc.tile_pool(name="pc_res", bufs=1) as pcr, \
             tc.tile_pool(name="pc_o", bufs=2) as pco, \
             tc.tile_pool(name="ps_o", bufs=1, space="PSUM") as ps_o, \
             tc.tile_pool(name="ps_s", bufs=2, space="PSUM") as ps_s, \
             tc.tile_pool(name="ps_r", bufs=1, space="PSUM") as ps_r, \
             tc.tile_pool(name="ps_y", bufs=1, space="PSUM") as ps_y:
            NP = JT // 2
            for ib in range(IB):
                po = ps_o.tile([128, CT, 512], F32)
                # rowsum halves share one accumulator: half-a is bounced
                # off-chip at mid-block, then the same bank restarts for
                # half-b (WAR dep on the copy serializes correctly)
                pr = ps_r.tile([1, 512], F32)
                # prefetch residual rows + bias for this i-block (one DMA)
                xrb = pcr.tile([128, 4, C], F32, tag="xrb")
                nc.sync.dma_start(
                    out=xrb,
                    in_=xq[ib * 512:(ib + 1) * 512, :].rearrange(
                        "(t p) c -> p t c", p=128))
                bpxs = []
                for ip in range(4):
                    bpx = pcr.tile([128, C], F32, tag=f"bpx{ip}")
                    nc.gpsimd.tensor_tensor(
                        bpx[:, :], xrb[:, ip, :], bp_bc2[:, :], OP.add)
                    bpxs.append(bpx)

                # software-pipelined j-loop: emit exps(n) BEFORE pv(n-1) and
                # scores(n+1) so the exp's program-order semaphore threshold
                # does not include the PV matmuls (which stalled the PE by
                # ~0.4us per iteration otherwise).
                def scores(n):
                    pair = []
                    for par in range(2):
                        j = 2 * n + par
                        pss = ps_s.tile([128, 512], F32, tag="scores")
                        for k2 in range(2):
                            nc.tensor.matmul(
                                pss[:, :],
                                kT[:, 2 * k2:2 * k2 + 2, j * 128:(j + 1) * 128],
                                qT[:, 2 * k2:2 * k2 + 2, ib * 512:(ib + 1) * 512],
                                start=(k2 == 0), stop=(k2 == 1), perf_mode=DR)
                        pair.append(pss)
                    return pair

                def exps(n, pair):
                    et = pcs.tile([128, 2, 512], F8, tag="exp")
                    for par in range(2):
                        nc.scalar.activation(et[:, par, :], pair[par], AF.Exp,
                                             bias=expb[:, :], scale=SM8)
                    return et

                def pv(n, et):
                    for ct in range(CT):
                        nc.tensor.matmul(
                            po[:, ct, :],
                            vS[:, 2 * n:2 * n + 2, ct * 128:(ct + 1) * 128],
                            et[:, :, :], start=(n == 0), stop=(n == NP - 1),
                            perf_mode=DR)
                    # row-sums of exp: 2.0^T @ etT -> [1, 512] (i on free
                    # dim), accumulated in per-half rows of pr
                    nc.tensor.matmul(
                        pr[:, :], ones2[:, :, 0:1], et[:, :, :],
                        start=(n % (NP // 2) == 0),
                        stop=(n % (NP // 2) == NP // 2 - 1), perf_mode=DR)

                def bounce(h):
                    srow = pcs.tile([1, 512], F32, tag=f"srow{h}")
                    nc.vector.tensor_copy(srow[:, :], pr[:, :])
                    nc.gpsimd.dma_start(out=sumscr[ib, h:h + 1, :],
                                        in_=srow[:, :])

                pair = scores(0)
                prev_et = None
                for n in range(NP):
                    et = exps(n, pair)
                    if n > 0:
                        pv(n - 1, prev_et)
                        if n - 1 == NP // 2 - 1:
                            bounce(0)   # first-half row-sums off-chip early
                    if n + 1 < NP:
                        pair = scores(n + 1)
                    prev_et = et
                pv(NP - 1, prev_et)
                bounce(1)
                # pull both halves back in per-partition layout [128, 4, 2],
                # add, then one cheap reciprocal
                st42 = pcr.tile([128, IB, 2], F32, tag="st42")
                for h in range(2):
                    nc.gpsimd.dma_start(
                        out=st42[:, :, h],
                        in_=sumscr[ib, h, :].rearrange("(b p) -> p b", p=128))
                st4 = pcr.tile([128, IB], F32, tag="st4")
                nc.vector.tensor_add(st4[:, :], st42[:, :, 0], st42[:, :, 1])
                rt = pcr.tile([128, IB], F32, tag="rt")
                nc.vector.reciprocal(rt[:, :], st4[:, :])
                # unnormalized outT eviction (scaled into fp8 range), split
                # scalar/vector so neither engine gates the projection
                ot = pco.tile([128, CT, 512], F8, tag="outT")
                for ct in range(CT):
                    if ct % 2 == 0:
                        nc.scalar.activation(ot[:, ct, :], po[:, ct, :],
                                             AF.Copy, scale=OTS)
                    else:
                        nc.vector.tensor_scalar(ot[:, ct, :], po[:, ct, :],
                                                OTS, None, OP.mult)
                # evict py to SBUF immediately (no rt dependency) so the four
                # projection groups stream through the single PSUM bank
                # without waiting on the row-sum bounce; normalize afterwards
                ycps = []
                for ip in range(4):
                    py = ps_y.tile([128, 512], F32, tag="proj")
                    for c2 in range(2):
                        nc.tensor.matmul(
                            py[:, :], ot[:, 2 * c2:2 * c2 + 2, ip * 128:(ip + 1) * 128],
                            wpr[:, 2 * c2:2 * c2 + 2, :],
                            start=(c2 == 0), stop=(c2 == 1), perf_mode=DR)
                    ycp = pcs.tile([128, C], F32, tag=f"ycp{ip}")
                    if ip % 2 == 0:
                        nc.vector.tensor_copy(ycp[:, :], py[:, :])
                    else:
                        nc.scalar.activation(ycp[:, :], py[:, :], AF.Copy)
                    ycps.append(ycp)
                for ip in range(4):
                    y2 = pcs.tile([128, C], BF16, tag="y2")
                    nc.vector.scalar_tensor_tensor(
                        y2[:, :], ycps[ip][:, :], rt[:, ip:ip + 1], bpxs[ip][:, :],
                        OP.mult, OP.add)
                    nc.sync.dma_start(out=yt[ib * 4 + ip, :, :], in_=y2[:, :])

        cst.release()
        store.release()

    if split_waits:
        _split_multi_waits(nc)
    return nc


_PROGRAM = None


def _get_program():
    global _PROGRAM
    if _PROGRAM is None:
        _PROGRAM = build_program()
    return _PROGRAM


def make_in_maps(x, gamma, beta, wq, bq, wk, bk, wv, bv, wp, bp):
    import ml_dtypes
    f32 = lambda a: np.ascontiguousarray(a, dtype=np.float32)
    # weights: x16, fp8e4m3, tiled [128, CT, C] with cin = ct*128 + p
    w8 = lambda a: np.ascontiguousarray(
        (f32(a) * WSC).astype(ml_dtypes.float8_e4m3)
        .reshape(CT, 128, C).transpose(1, 0, 2))
    xr = f32(x).reshape(B, HW, C)
    gsel = np.zeros((GROUPS, C), dtype=np.float32)
    for g in range(GROUPS):
        gsel[g, g * GSIZE:(g + 1) * GSIZE] = 1.0
    gsel2 = np.zeros((128, CT, GROUPS), dtype=np.float32)
    for p in range(128):
        for ct in range(CT):
            gsel2[p, ct, (ct * 128 + p) // GSIZE] = 1.0 / GSIZE
    # packed per-channel constants: cvec[p, ct, :] = (16bq, 16bk, gamma, beta)
    cvec = np.stack([f32(bq) * WSC, f32(bk) * WSC, f32(gamma), f32(beta)],
                    axis=1).reshape(CT, 128, 4).transpose(1, 0, 2)
    common = {
        "wq": w8(wq), "wk": w8(wk), "wv": w8(wv), "wp": w8(wp),
        "cvec": np.ascontiguousarray(cvec),
        # bv rides through attention (softmax rows sum to 1): fold into bp
        "bp": f32(bp) + f32(bv) @ f32(wp),
        "gsel": gsel, "gsel2": gsel2,
    }
    in_maps = []
    for c in range(N_CORES):
        b, h = c // 2, c % 2
        m = dict(common)
        if h == 0:
            xp = xr[b]
        else:
            xp = np.concatenate([xr[b, HALF:], xr[b, :HALF]], axis=0)
        # pre-transpose to channel-major, slab-tiled [2, CT, 128, 2048] fp8
        # so each (half, ct) slab DMA is one contiguous 256KB read
        m["xT"] = np.ascontiguousarray(
            xp.T.astype(ml_dtypes.float8_e4m3).reshape(CT, 128, 2, 2048)
            .transpose(2, 0, 1, 3))
        m["xq"] = np.ascontiguousarray(xr[b, h * HALF:(h + 1) * HALF])
        in_maps.append(m)
    return in_maps


def kernel(x, gamma, beta, wq, bq, wk, bk, wv, bv, wp, bp, _trace=False):
    nc = _get_program()
    in_maps = make_in_maps(x, gamma, beta, wq, bq, wk, bk, wv, bv, wp, bp)
    res = run_bass_kernel_spmd(nc, in_maps, list(range(N_CORES)), trace=_trace)
    out = np.empty((B, HW, C), dtype=np.float32)
    for c in range(N_CORES):
        b, h = c // 2, c % 2
        out[b, h * HALF:(h + 1) * HALF] = np.asarray(
            res.results[c]["y"], dtype=np.float32)
    if _trace:
        kernel._last_result = res
    return out.reshape(B, H, W, C)



# revision 39
# speedup vs baseline: 1.0483x; 1.0483x over previous
"""AttentionBlock (GroupNorm + single-head 4096x4096 attention + proj + residual)
on 8 Trainium2 NeuronCores.

Sharding: core c = 2*b + h handles image b (of 4), query-half h (of 2).
Each core:
  - receives x pre-transposed to channel-major bf16 (host does the transpose),
  - computes GroupNorm statistics via bn_stats while x streams in,
  - computes kT [512,4096] and v [4096,512] for the full image (k/v duplicated
    across the half-pair, ~10% extra FLOPs, no collectives),
  - computes qT for its 2048 query rows,
  - attention over its 2048 queries, projection + bias + residual for its rows.

Precision: fp8e4m3 (TRN 240-max) with MatmulPerfMode.DoubleRow for ALL GEMMs
(hn/q/k/v/scores/PV/rowsum/proj), fp32 PSUM accumulation, fp32 GroupNorm
statistics, fp32 softmax row-sums / normalization, fp32 residual.

fp8 scale bookkeeping:
  - wq/wk/wv/wp host-scaled x16 (keeps N(0, 1/sqrt(C)) weights out of the fp8
    subnormal range); biases bq/bk/bv host-scaled x16 to match.
  - qT/kT hold 16q/16k; scores PSUM = 256*q.k; exp scale folds the 1/256.
  - exp has bias -2.0 (max score 6.81 -> et max e^4.81=123 < 240 fp8 max);
    the e^-2 factor cancels between numerator and row-sum.
  - vS holds 16v; po PSUM = 16*sum(et*v); ot eviction scales by 1/128.
  - proj PSUM py = (po/128) @ (16 wp) = 2*sum(et*v)@wp; rowsum matmul uses a
    2.0-valued ones vector so rt = 1/pr = 1/(2*sum(et)) normalizes exactly.
"""

import sys

sys.path.insert(0, "/opt/trn_rl_repo")

import numpy as np  # noqa: E402

import bass_rust  # noqa: E402
import concourse.bass as bass  # noqa: E402
import concourse.mybir as mybir  # noqa: E402
import concourse.tile as tile  # noqa: E402
from concourse.vector_clock import ScopedClock  # noqa: E402
from concourse.bass_utils import run_bass_kernel_spmd  # noqa: E402

F32 = mybir.dt.float32
BF16 = mybir.dt.bfloat16
F8 = mybir.dt.float8e4
AF = mybir.ActivationFunctionType
OP = mybir.AluOpType
DR = mybir.MatmulPerfMode.DoubleRow

B, H, W, C = 4, 64, 64, 512
HW = H * W            # 4096 positions per image
HALF = HW // 2        # 2048 query rows per core
GROUPS = 32
GSIZE = C // GROUPS   # 16 channels per group
EPS = 1e-5
N_CORES = 8
CT = C // 128         # 4 channel partition-tiles
JT = HW // 128        # 32 position partition-tiles
JC = HW // 512        # 8 position chunks (kT/v build)
QC = HALF // 512      # 4 query chunks (qT build)
IB = HALF // 512      # 4 query i-blocks (attention)
WSC = 16.0            # host-side weight/bias scale (fp8 subnormal avoidance)
SM8 = 1.0 / (WSC * WSC * float(np.sqrt(C)))   # exp scale on (16q).(16k) psum
EXPB = -2.0           # exp bias: keeps et = e^(s-2) <= e^4.9 < 240 (fp8 max)
OTS = 1.0 / 128.0     # po -> ot eviction scale (fp8 range)
ONESV = 2.0           # rowsum weights: pr = 2*sum(et) so rt=1/pr normalizes
                      # py = (po/128)@(16wp) = 2*sum(et*v)@wp exactly


# --- workaround: walrus in this container rejects instructions carrying more
# than one sync-wait command.  Move extra waits onto same-engine NOPs placed
# immediately before the instruction (engine program order makes this exact).
def _split_multi_waits(nc, max_waits=1):
    n = 0
    for f in nc.m.functions:
        for bb in f.blocks:
            newlist = []
            for inst in bb.instructions:
                si = inst.sync_info
                waits = list(si.on_wait) if si is not None else []
                if len(waits) > max_waits:
                    n += 1
                    for k, wt in enumerate(waits[:-max_waits]):
                        nop = bass_rust.InstNoOp(
                            name=f"{inst.name}-sw{k}", engine=inst.engine)
                        nop.sync_info = mybir.SyncInfo(on_wait=[wt], on_update=[])
                        newlist.append(nop)
                    inst.sync_info = mybir.SyncInfo(
                        on_wait=waits[-max_waits:], on_update=list(si.on_update))
                newlist.append(inst)
            bb.instructions[:] = newlist
    return n


def _split_drain_and_barrier(self, tick_clock, wait_clock):
    # same as TileContext._drain_and_barrier but with the tail drain's waits
    # split onto single-wait NOPs (same walrus limitation as above).
    drain_inst = self.nc.sync.drain()
    wait_clock.add_sem_waits(
        drain_inst.ins, ScopedClock({None: tick_clock.global_clock}))
    mi = drain_inst.ins
    waits = list(mi.sync_info.on_wait) if mi.sync_info is not None else []
    if len(waits) > 1:
        mi.sync_info.on_wait = []
        for wt in waits:
            wi = self.nc.sync.nop(nofuse=True, hint="tail_drain_wait")
            wi.ins.sync_info = mybir.SyncInfo(on_wait=[wt], on_update=[])
    self.nc.all_engine_barrier()
    assert self.sems is not None
    popped = self.nc._tile_sem_poison_stack.pop()
    assert popped is self._sem_poison
    self.nc.clear_and_free_semaphores(list(self.sems.allocated().values()))
    self.nc.all_engine_barrier()


tile.TileContext._drain_and_barrier = _split_drain_and_barrier


def build_program(split_waits=True):
    nc = bass.Bass()

    # xT rows are permuted per-core so the query half is always positions
    # [0, HALF), host-transposed to channel-major and slab-tiled
    # [2, CT, 128, 2048] fp8 so each (half, ct) slab DMA is one contiguous
    # 256KB read -- the x stream gates the GroupNorm stats chain, so halving
    # its bytes (vs bf16) shortens the serial startup.  fp8 x only perturbs
    # hn by ~3% (already the hnT quantization level) and GN variance by 0.1%.
    xTd = nc.dram_tensor("xT", [2, CT, 128, 2048], F8, kind="ExternalInput")
    xq = nc.dram_tensor("xq", [HALF, C], F32, kind="ExternalInput")
    # weights pre-cast to fp8 (x16) and pre-tiled [128, CT, C] on the host --
    # one fewer rounding than the old bf16->fp8 device cast, half the DMA
    # bytes, and no scalar-engine casts clogging the queue ahead of the GN
    # sqrt (which stalled phase B by ~7us).
    wq = nc.dram_tensor("wq", [128, CT, C], F8, kind="ExternalInput")
    wk = nc.dram_tensor("wk", [128, CT, C], F8, kind="ExternalInput")
    wv = nc.dram_tensor("wv", [128, CT, C], F8, kind="ExternalInput")
    wp = nc.dram_tensor("wp", [128, CT, C], F8, kind="ExternalInput")
    # packed per-channel constants [128, CT, 4] = (16*bq, 16*bk, gamma, beta)
    cvecd = nc.dram_tensor("cvec", [128, CT, 4], F32, kind="ExternalInput")
    # bp here is host-computed bp + bv @ wp (bv folded through the attention)
    bpd = nc.dram_tensor("bp", [C], F32, kind="ExternalInput")
    gseld = nc.dram_tensor("gsel", [GROUPS, C], F32, kind="ExternalInput")
    # gsel2[p, ct, g] = 1/GSIZE where channel ct*128+p belongs to group g
    gsel2d = nc.dram_tensor("gsel2", [128, CT, GROUPS], F32, kind="ExternalInput")
    # y in bf16: halves the output DMA on the tail critical path; the bf16
    # rounding of x + proj adds <= 2^-9 relative error (~2e-3, budget 2e-2)
    yd = nc.dram_tensor("y", [HALF, C], BF16, kind="ExternalOutput")

    xqt = xq[:, :].rearrange("(t p) c -> t p c", p=128)   # [16,128,512]
    yt = yd[:, :].rearrange("(t p) c -> t p c", p=128)    # [16,128,512]

    with tile.TileContext(nc) as tc:
        # ---------------- persistent storage + constants ----------------
        store = tc.alloc_tile_pool(name="store", bufs=1)
        kT = store.tile([128, CT, HW], F8)       # kT[c%128, c//128, j] = 16k
        vS = store.tile([128, JT, C], F8)        # v[j%128, j//128, c] = 16v
        qT = store.tile([128, CT, HALF], F8)     # qT[c%128, c//128, i] = 16q
        # x^T in fp8: one tile [cin%128, cin//128, half, j-in-half] so QKV
        # matmul moving APs can span ct-pairs (DoubleRow contraction)
        xTa = store.tile([128, CT, 2, 2048], F8)
        wqr = store.tile([128, CT, C], F8)       # 16*wq, [cin%128, cin//128, cout]
        wkr = store.tile([128, CT, C], F8)
        wvr = store.tile([128, CT, C], F8)
        wpr = store.tile([128, CT, C], F8)
        # GN-scale-folded weights: wS = diag(s) @ (16*w), built in phase A
        wqS = store.tile([128, CT, C], F8)
        wkS = store.tile([128, CT, C], F8)
        wvS = store.tile([128, CT, C], F8)
        # 256*t and 256*(t^T wv) as DR weights (col 0; pair-dim step 16B)
        t8f = store.tile([128, CT, 16], F8)
        twv8f = store.tile([128, CT, 16], F8)
        cst = tc.alloc_tile_pool(name="cst", bufs=1)
        gsel = cst.tile([GROUPS, C], F32)
        gsel2 = cst.tile([128, CT, GROUPS], F32)
        # [128, 2, 16] so the DoubleRow weight AP's pair-dim step is 16 bytes
        # (walrus s3_lw_dual_fp8_restrictions requires step % 16 == 0)
        ones2 = cst.tile([128, 2, 16], F8)
        nc.vector.memset(ones2, ONESV)
        expb = cst.tile([128, 1], F32)
        nc.vector.memset(expb, EXPB)
        # DRAM scratch to re-layout softmax row-sums [1,512] -> [128,4]
        # (two halves per i-block: the n<8 half bounces mid-block so only
        # the second half's round-trip sits on the tail critical path)
        sumscr = nc.dram_tensor("sumscr", [IB, 2, 512], F32)
        # DRAM scratch to re-layout t^T w rank-1 results [1,512] -> [128,4]
        twscr = nc.dram_tensor("twscr", [4, 512], F32)
        cv = cst.tile([128, CT, 4], F32)   # (16bq, 16bk, gamma, beta)
        bp_bc = cst.tile([128, C], F32)
        s_sb = cst.tile([128, CT], F32)   # GN scale per channel
        t_sb = cst.tile([128, CT], F32)   # GN shift per channel
        # eviction biases with the GN t-term folded in: 16(bq + t^T wq) etc.
        cvq2 = cst.tile([128, CT], F32)
        bp_bc2 = cst.tile([128, C], F32)   # bp + bv@wp + (t^T wv)@wp

        # 8 x-slab DMAs (512KB contiguous each) FIRST, spread over the three
        # DMA-capable queues (two queues topped out at ~200GB/s; 4MB of x
        # gates the GroupNorm stats chain), ct-major so the per-ct stats
        # aggregation pipelines behind the DMAs.  Weights + consts after.
        xengs = [nc.sync, nc.gpsimd, nc.scalar]
        for ct in range(CT):
            for hf in range(2):
                xengs[(2 * ct + hf) % 3].dma_start(
                    out=xTa[:, ct, hf, :], in_=xTd[hf, ct, :, :])
        nc.sync.dma_start(out=gsel2, in_=gsel2d[:, :, :])
        nc.sync.dma_start(out=cv, in_=cvecd[:, :, :])
        nc.sync.dma_start(out=wkr, in_=wk[:, :, :])
        nc.sync.dma_start(out=wpr, in_=wp[:, :, :])
        nc.scalar.dma_start(out=gsel, in_=gseld[:, :])
        nc.scalar.dma_start(out=bp_bc, in_=bpd[:].partition_broadcast(128))
        nc.scalar.dma_start(out=wvr, in_=wv[:, :, :])
        nc.gpsimd.dma_start(out=wqr, in_=wq[:, :, :])

        # ------- phase A: GroupNorm stats as the slabs land (no PE work) ----
        # ct-major: each ct's stats aggregate + feed the group matmul while
        # the next ct's slabs are still streaming in, shortening the serial
        # chain after the last bn_stats.
        with tc.tile_pool(name="pa_small", bufs=1) as pas, \
             tc.tile_pool(name="pa_ps", bufs=2, space="PSUM") as pa_ps:
            stats_sb = pas.tile([128, CT, JC, 6], F32)
            # full-array DR warm weights, written AT the stats-chain marker
            # points (data-dependence gates each warm block's start)
            wgates = [pas.tile([128, 2, 128], F8, tag=f"wg{i}", name=f"wg{i}")
                      for i in range(3)]
            epst = pas.tile([GROUPS, 1], F32)
            nc.vector.memset(epst, EPS)
            g2 = pa_ps.tile([GROUPS, 2], F32, tag="gagg")
            mv_all = pas.tile([128, CT, 2], F32)
            sp_all = pas.tile([128, CT, 2], F32)
            for ct in range(CT):
                for jc in range(JC):
                    hf, sc = jc // 4, (jc % 4) * 512
                    nc.vector.bn_stats(
                        out=stats_sb[:, ct, jc, :],
                        in_=xTa[:, ct, hf, sc:sc + 512])
                    if ct == 0 and jc == 0:
                        # gate warm block 0 on the very first stats tile
                        nc.vector.tensor_scalar(
                            wgates[0][:, :, :], xTa[:, 0:2, 0, 0:128],
                            stats_sb[:, 0, 0, 1:2], None, OP.mult)
                nc.vector.bn_aggr(out=mv_all[:, ct, :], in_=stats_sb[:, ct, :, :])
                nc.vector.tensor_mul(sp_all[:, ct, 0:1], mv_all[:, ct, 0:1],
                                     mv_all[:, ct, 0:1])
                nc.vector.tensor_add(sp_all[:, ct, 1:2], sp_all[:, ct, 0:1],
                                     mv_all[:, ct, 1:2])
                nc.vector.tensor_copy(sp_all[:, ct, 0:1], mv_all[:, ct, 0:1])
                nc.tensor.matmul(g2[:, :], gsel2[:, ct, :], sp_all[:, ct, :],
                                 start=(ct == 0), stop=(ct == CT - 1))
                if ct == 2:
                    # mid-chain marker; warm matmuls MUST be full-array: HAM
                    # gates the clock on PE array power, and a 1-column warm
                    # matmul reads as idle, re-throttling to 1.2GHz right
                    # before phase B.
                    nc.vector.tensor_scalar(
                        wgates[1][:, :, :], xTa[:, 0:2, 0, 0:128],
                        stats_sb[:, ct, 0, 1:2], None, OP.mult)
            nc.vector.tensor_scalar(
                wgates[2][:, :, :], xTa[:, 0:2, 0, 0:128],
                stats_sb[:, 3, 0, 1:2], None, OP.mult)
            with tc.tile_pool(name="pa_warm", bufs=1, space="PSUM") as pwm:
                pw = pwm.tile([128, 512], F32)
                for wi, reps in ((0, 48), (1, 18), (2, 18)):
                    for r in range(reps):
                        nc.tensor.matmul(
                            pw[:, :], wgates[wi][:, :, :],
                            xTa[:, 0:2, 0, 0:512],
                            start=True, stop=True, perf_mode=DR)

            if True:
                # group mean/var -> (mean, rstd); keep the serial chain on the
                # vector engine (one scalar hop for sqrt) -- cross-engine hops
                # cost ~0.5-1us each in queue + semaphore latency
                mv2 = pas.tile([GROUPS, 2], F32)
                nc.vector.tensor_copy(mv2[:, :], g2[:, :])   # (mean, E[x^2])
                var = pas.tile([GROUPS, 1], F32)
                nc.vector.tensor_mul(var[:, :], mv2[:, 0:1], mv2[:, 0:1])
                nc.vector.tensor_sub(var[:, :], mv2[:, 1:2], var[:, :])
                sd = pas.tile([GROUPS, 1], F32)
                nc.scalar.activation(sd[:, :], var[:, :], AF.Sqrt, bias=epst[:, :])
                nc.vector.reciprocal(mv2[:, 1:2], sd[:, :])
                # broadcast group (mean, rstd) to channels into ONE psum tile,
                # one eviction, then s/t (batched on vector)
                bc_all = pas.tile([128, CT, 2], F32)
                pbc = pa_ps.tile([128, CT, 2], F32, tag="bcast")
                for ct in range(CT):
                    nc.tensor.matmul(pbc[:, ct, :], gsel[:, ct * 128:(ct + 1) * 128],
                                     mv2[:, :], start=True, stop=True)
                nc.vector.tensor_copy(bc_all[:, :, :], pbc[:, :, :])
                nc.vector.tensor_mul(s_sb[:, :], cv[:, :, 2], bc_all[:, :, 1])
                tmp = pas.tile([128, CT], F32)
                nc.vector.tensor_mul(tmp[:, :], bc_all[:, :, 0], s_sb[:, :])
                nc.vector.tensor_sub(t_sb[:, :], cv[:, :, 3], tmp[:, :])
                # fold the GN scale into the QKV weights (wS = diag(s) @ 16w):
                # k = (diag(s) 16wk)^T x + 16(t^T wk + bk).  This removes the
                # per-position hn normalization pass entirely -- phase B's
                # GEMMs consume the raw fp8 x slabs.  Split across the three
                # elementwise engines; wkS first (kT GEMMs consume it first).
                # 256*t as a DR weight column (rank-1 t^T w matmuls below;
                # first on vector -- the t^T w matmuls are the PE's first
                # real work and gate the eviction-bias chain)
                nc.vector.tensor_scalar(t8f[:, :, 0], t_sb[:, :],
                                        256.0, None, OP.mult)
                # gpsimd lacks TensorScalarPtr, so the 12 scale ops split
                # vector/scalar in consumption order (K, Q, then V); keeping
                # vector's tail short matters -- phase B's PSUM evictions
                # queue behind it, and a clogged vector queue starves the PE
                # of free PSUM banks.
                for ct in range(CT):
                    nc.vector.tensor_scalar(wkS[:, ct, :], wkr[:, ct, :],
                                            s_sb[:, ct:ct + 1], None, OP.mult)
                    nc.scalar.activation(wqS[:, ct, :], wqr[:, ct, :],
                                         AF.Copy, scale=s_sb[:, ct:ct + 1])
                for ct in range(CT):
                    if ct < 2:
                        nc.vector.tensor_scalar(wvS[:, ct, :], wvr[:, ct, :],
                                                s_sb[:, ct:ct + 1], None,
                                                OP.mult)
                    else:
                        nc.scalar.activation(wvS[:, ct, :], wvr[:, ct, :],
                                             AF.Copy,
                                             scale=s_sb[:, ct:ct + 1])

        # ---------------- phase B: K,V (and Q) GEMMs on raw x ---------------
        # eviction engines alternate scalar/vector (gpsimd cannot read PSUM)
        _ev = [0]

        def ev_out(out, pin, bias):
            e = _ev[0] % 2
            _ev[0] += 1
            if bias is None:
                if e == 0:
                    nc.scalar.activation(out, pin, AF.Copy)
                else:
                    nc.vector.tensor_copy(out, pin)
            else:
                if e == 0:
                    nc.scalar.activation(out, pin, AF.Identity, bias=bias)
                else:
                    nc.vector.tensor_scalar(out, pin, bias, None, OP.add)

        def qkv_chunk(pb_ps, jc):
            hf, js = jc // 4, (jc % 4) * 512
            for ct in range(CT):
                pk = pb_ps.tile([128, 512], F32, tag="qkv")
                for k2 in range(2):
                    nc.tensor.matmul(
                        pk[:, :], wkS[:, 2 * k2:2 * k2 + 2, ct * 128:(ct + 1) * 128],
                        xTa[:, 2 * k2:2 * k2 + 2, hf, js:js + 512],
                        start=(k2 == 0), stop=(k2 == 1), perf_mode=DR)
                # no k bias: softmax_j(q_i.(k_j + c)) == softmax_j(q_i.k_j)
                # -- any per-channel key offset (bk and the GN t-term) adds a
                # j-constant q_i.c to row i's scores and cancels exactly
                ev_out(kT[:, ct, jc * 512:(jc + 1) * 512], pk[:, :], None)
            for jp in range(4):
                pv = pb_ps.tile([128, 512], F32, tag="qkv")
                for k2 in range(2):
                    nc.tensor.matmul(
                        pv[:, :],
                        xTa[:, 2 * k2:2 * k2 + 2, hf, js + jp * 128:js + (jp + 1) * 128],
                        wvS[:, 2 * k2:2 * k2 + 2, :],
                        start=(k2 == 0), stop=(k2 == 1), perf_mode=DR)
                # bv is folded into bp on the host (softmax rows sum to 1 so
                # attn(v + bv) = attn(v) + bv exactly); the GN t-term rides
                # the same way via bp_bc2.  Eviction is a pure cast.
                ev_out(vS[:, jc * 4 + jp, :], pv[:, :], None)
            if jc < QC:   # rows [0, HALF) are the query rows
                for ct in range(CT):
                    pq = pb_ps.tile([128, 512], F32, tag="qkv")
                    for k2 in range(2):
                        nc.tensor.matmul(
                            pq[:, :], wqS[:, 2 * k2:2 * k2 + 2, ct * 128:(ct + 1) * 128],
                            xTa[:, 2 * k2:2 * k2 + 2, hf, js:js + 512],
                            start=(k2 == 0), stop=(k2 == 1), perf_mode=DR)
                    ev_out(qT[:, ct, jc * 512:(jc + 1) * 512], pq[:, :],
                           cvq2[:, ct:ct + 1])

        with tc.tile_pool(name="pb_ps", bufs=7, space="PSUM") as pb_ps, \
             tc.tile_pool(name="tw_ps", bufs=1, space="PSUM") as tw_ps, \
             tc.tile_pool(name="tw_sb", bufs=4) as tw_sb:
            # rank-1 t^T w matmuls (one 216ns matmul each; evict via scalar
            # copy -> DRAM bounce -> [128, CT] relayout on the gpsimd queue),
            # interleaved with the first QKV chunks so the PE never idles.
            def twmm(wr, row):
                ptw = tw_ps.tile([1, 512], F32, tag="tw")
                for k2 in range(2):
                    nc.tensor.matmul(
                        ptw[:, :], t8f[:, 2 * k2:2 * k2 + 2, 0:1],
                        wr[:, 2 * k2:2 * k2 + 2, :],
                        start=(k2 == 0), stop=(k2 == 1), perf_mode=DR)
                stg = tw_sb.tile([1, 512], F32, tag=f"twe{row}")
                nc.scalar.activation(stg[:, :], ptw[:, :], AF.Copy)
                nc.gpsimd.dma_start(out=twscr[row:row + 1, :], in_=stg[:, :])

            def twback(row, out4):
                nc.gpsimd.dma_start(
                    out=out4,
                    in_=twscr[row, :].rearrange("(b p) -> p b", p=128))

            twmm(wqr, 0)
            twmm(wvr, 1)
            twq4 = cst.tile([128, CT], F32)
            twv4 = cst.tile([128, CT], F32)
            twback(0, twq4[:, :])
            # 16(bq + t^T wq) = 16bq + psum/256   (psum = 256t . 16wq)
            nc.vector.scalar_tensor_tensor(
                cvq2[:, :], twq4[:, :], 1.0 / 256.0, cv[:, :, 0],
                OP.mult, OP.add)
            qkv_chunk(pb_ps, 0)
            twback(1, twv4[:, :])
            # 256*(t^T wv) in fp8 for the second rank-1 hop through wp
            nc.vector.tensor_scalar(twv8f[:, :, 0], twv4[:, :],
                                    1.0 / 16.0, None, OP.mult)
            qkv_chunk(pb_ps, 1)
            qkv_chunk(pb_ps, 2)
            # (t^T wv) @ wp -> broadcast into bp_bc2 (rides the softmax like
            # bv: attn rows sum to 1, so it lands as a per-cout constant)
            ptv = tw_ps.tile([1, 512], F32, tag="tw")
            for k2 in range(2):
                nc.tensor.matmul(
                    ptv[:, :], twv8f[:, 2 * k2:2 * k2 + 2, 0:1],
                    wpr[:, 2 * k2:2 * k2 + 2, :],
                    start=(k2 == 0), stop=(k2 == 1), perf_mode=DR)
            stv = tw_sb.tile([1, 512], F32, tag="twe3")
            nc.scalar.activation(stv[:, :], ptv[:, :], AF.Copy)
            nc.gpsimd.dma_start(out=twscr[2:3, :], in_=stv[:, :])
            tvpb = tw_sb.tile([128, C], F32, tag="tvpb")
            nc.gpsimd.dma_start(
                out=tvpb[:, :], in_=twscr[2, :].partition_broadcast(128))
            nc.vector.scalar_tensor_tensor(
                bp_bc2[:, :], tvpb[:, :], 1.0 / 4096.0, bp_bc[:, :],
                OP.mult, OP.add)
            for jc in range(3, JC):
                qkv_chunk(pb_ps, jc)

        # ---------------- phase C: attention + projection + residual --------
        with tc.tile_pool(name="pc_sb", bufs=4) as pcs, \
             tc.tile_pool(name="pc_res", bufs=1) as pcr, \
             tc.tile_pool(name="pc_o", bufs=2) as pco, \
             tc.tile_pool(name="ps_o", bufs=1, space="PSUM") as ps_o, \
             tc.tile_pool(name="ps_s", bufs=2, space="PSUM") as ps_s, \
             tc.tile_pool(name="ps_r", bufs=1, space="PSUM") as ps_r, \
             tc.tile_pool(name="ps_y", bufs=1, space="PSUM") as ps_y:
            NP = JT // 2
            for ib in range(IB):
                po = ps_o.tile([128, CT, 512], F32)
                # rowsum halves share one accumulator: half-a is bounced
                # off-chip at mid-block, then the same bank restarts for
                # half-b (WAR dep on the copy serializes correctly)
                pr = ps_r.tile([1, 512], F32)
                # prefetch residual rows + bias for this i-block (one DMA)
                xrb = pcr.tile([128, 4, C], F32, tag="xrb")
                nc.sync.dma_start(
                    out=xrb,
                    in_=xq[ib * 512:(ib + 1) * 512, :].rearrange(
                        "(t p) c -> p t c", p=128))
                bpxs = []
                for ip in range(4):
                    bpx = pcr.tile([128, C], F32, tag=f"bpx{ip}")
                    nc.gpsimd.tensor_tensor(
                        bpx[:, :], xrb[:, ip, :], bp_bc2[:, :], OP.add)
                    bpxs.append(bpx)

                # software-pipelined j-loop: emit exps(n) BEFORE pv(n-1) and
                # scores(n+1) so the exp's program-order semaphore threshold
                # does not include the PV matmuls (which stalled the PE by
                # ~0.4us per iteration otherwise).
                def scores(n):
                    pair = []
                    for par in range(2):
                        j = 2 * n + par
                        pss = ps_s.tile([128, 512], F32, tag="scores")
                        for k2 in range(2):
                            nc.tensor.matmul(
                                pss[:, :],
                                kT[:, 2 * k2:2 * k2 + 2, j * 128:(j + 1) * 128],
                                qT[:, 2 * k2:2 * k2 + 2, ib * 512:(ib + 1) * 512],
                                start=(k2 == 0), stop=(k2 == 1), perf_mode=DR)
                        pair.append(pss)
                    return pair

                def exps(n, pair):
                    et = pcs.tile([128, 2, 512], F8, tag="exp")
                    for par in range(2):
                        nc.scalar.activation(et[:, par, :], pair[par], AF.Exp,
                                             bias=expb[:, :], scale=SM8)
                    return et

                def pv(n, et):
                    for ct in range(CT):
                        nc.tensor.matmul(
                            po[:, ct, :],
                            vS[:, 2 * n:2 * n + 2, ct * 128:(ct + 1) * 128],
                            et[:, :, :], start=(n == 0), stop=(n == NP - 1),
                            perf_mode=DR)
                    # row-sums of exp: 2.0^T @ etT -> [1, 512] (i on free
                    # dim), accumulated in per-half rows of pr
                    nc.tensor.matmul(
                        pr[:, :], ones2[:, :, 0:1], et[:, :, :],
                        start=(n % (NP // 2) == 0),
                        stop=(n % (NP // 2) == NP // 2 - 1), perf_mode=DR)

                def bounce(h):
                    srow = pcs.tile([1, 512], F32, tag=f"srow{h}")
                    nc.vector.tensor_copy(srow[:, :], pr[:, :])
                    nc.gpsimd.dma_start(out=sumscr[ib, h:h + 1, :],
                                        in_=srow[:, :])

                pair = scores(0)
                prev_et = None
                for n in range(NP):
                    et = exps(n, pair)
                    if n > 0:
                        pv(n - 1, prev_et)
                        if n - 1 == NP // 2 - 1:
                            bounce(0)   # first-half row-sums off-chip early
                    if n + 1 < NP:
                        pair = scores(n + 1)
                    prev_et = et
                pv(NP - 1, prev_et)
                bounce(1)
                # pull both halves back in per-partition layout [128, 4, 2],
                # add, then one cheap reciprocal
                st42 = pcr.tile([128, IB, 2], F32, tag="st42")
                for h in range(2):
                    nc.gpsimd.dma_start(
                        out=st42[:, :, h],
                        in_=sumscr[ib, h, :].rearrange("(b p) -> p b", p=128))
                st4 = pcr.tile([128, IB], F32, tag="st4")
                nc.vector.tensor_add(st4[:, :], st42[:, :, 0], st42[:, :, 1])
                rt = pcr.tile([128, IB], F32, tag="rt")
                nc.vector.reciprocal(rt[:, :], st4[:, :])
                # unnormalized outT eviction (scaled into fp8 range), split
                # scalar/vector so neither engine gates the projection
                ot = pco.tile([128, CT, 512], F8, tag="outT")
                for ct in range(CT):
                    if ct % 2 == 0:
                        nc.scalar.activation(ot[:, ct, :], po[:, ct, :],
                                             AF.Copy, scale=OTS)
                    else:
                        nc.vector.tensor_scalar(ot[:, ct, :], po[:, ct, :],
                                                OTS, None, OP.mult)
                # evict py to SBUF immediately (no rt dependency) so the four
                # projection groups stream through the single PSUM bank
                # without waiting on the row-sum bounce; normalize afterwards
                ycps = []
                for ip in range(4):
                    py = ps_y.tile([128, 512], F32, tag="proj")
                    for c2 in range(2):
                        nc.tensor.matmul(
                            py[:, :], ot[:, 2 * c2:2 * c2 + 2, ip * 128:(ip + 1) * 128],
                            wpr[:, 2 * c2:2 * c2 + 2, :],
                            start=(c2 == 0), stop=(c2 == 1), perf_mode=DR)
                    ycp = pcs.tile([128, C], F32, tag=f"ycp{ip}")
                    if ip % 2 == 0:
                        nc.vector.tensor_copy(ycp[:, :], py[:, :])
                    else:
                        nc.scalar.activation(ycp[:, :], py[:, :], AF.Copy)
                    ycps.append(ycp)
                for ip in range(4):
                    y2 = pcs.tile([128, C], BF16, tag="y2")
                    nc.vector.scalar_tensor_tensor(
                        y2[:, :], ycps[ip][:, :], rt[:, ip:ip + 1], bpxs[ip][:, :],
                        OP.mult, OP.add)
                    nc.sync.dma_start(out=yt[ib * 4 + ip, :, :], in_=y2[:, :])

        cst.release()
        store.release()

    if split_waits:
        _split_multi_waits(nc)
    return nc


_PROGRAM = None


def _get_program():
    global _PROGRAM
    if _PROGRAM is None:
        _PROGRAM = build_program()
    return _PROGRAM


def make_in_maps(x, gamma, beta, wq, bq, wk, bk, wv, bv, wp, bp):
    import ml_dtypes
    f32 = lambda a: np.ascontiguousarray(a, dtype=np.float32)
    # weights: x16, fp8e4m3, tiled [128, CT, C] with cin = ct*128 + p
    w8 = lambda a: np.ascontiguousarray(
        (f32(a) * WSC).astype(ml_dtypes.float8_e4m3)
        .reshape(CT, 128, C).transpose(1, 0, 2))
    xr = f32(x).reshape(B, HW, C)
    gsel = np.zeros((GROUPS, C), dtype=np.float32)
    for g in range(GROUPS):
        gsel[g, g * GSIZE:(g + 1) * GSIZE] = 1.0
    gsel2 = np.zeros((128, CT, GROUPS), dtype=np.float32)
    for p in range(128):
        for ct in range(CT):
            gsel2[p, ct, (ct * 128 + p) // GSIZE] = 1.0 / GSIZE
    # packed per-channel constants: cvec[p, ct, :] = (16bq, 16bk, gamma, beta)
    cvec = np.stack([f32(bq) * WSC, f32(bk) * WSC, f32(gamma), f32(beta)],
                    axis=1).reshape(CT, 128, 4).transpose(1, 0, 2)
    common = {
        "wq": w8(wq), "wk": w8(wk), "wv": w8(wv), "wp": w8(wp),
        "cvec": np.ascontiguousarray(cvec),
        # bv rides through attention (softmax rows sum to 1): fold into bp
        "bp": f32(bp) + f32(bv) @ f32(wp),
        "gsel": gsel, "gsel2": gsel2,
    }
    in_maps = []
    for c in range(N_CORES):
        b, h = c // 2, c % 2
        m = dict(common)
        if h == 0:
            xp = xr[b]
        else:
            xp = np.concatenate([xr[b, HALF:], xr[b, :HALF]], axis=0)
        # pre-transpose to channel-major, slab-tiled [2, CT, 128, 2048] fp8
        # so each (half, ct) slab DMA is one contiguous 256KB read
        m["xT"] = np.ascontiguousarray(
            xp.T.astype(ml_dtypes.float8_e4m3).reshape(CT, 128, 2, 2048)
            .transpose(2, 0, 1, 3))
        m["xq"] = np.ascontiguousarray(xr[b, h * HALF:(h + 1) * HALF])
        in_maps.append(m)
    return in_maps


def kernel(x, gamma, beta, wq, bq, wk, bk, wv, bv, wp, bp, _trace=False):
    nc = _get_program()
    in_maps = make_in_maps(x, gamma, beta, wq, bq, wk, bk, wv, bv, wp, bp)
    res = run_bass_kernel_spmd(nc, in_maps, list(range(N_CORES)), trace=_trace)
    out = np.empty((B, HW, C), dtype=np.float32)
    for c in range(N_CORES):
        b, h = c // 2, c % 2
        out[b, h * HALF:(h + 1) * HALF] = np.asarray(
            res.results[c]["y"], dtype=np.float32)
    if _trace:
        kernel._last_result = res
    return out.reshape(B, H, W, C)

